# revision 1
# baseline (speedup 1.0000x reference)
"""Trainium2 Bass kernel v3 for PhysicsInformedNN (Navier-Stokes PINN).

Jet propagation with 11 channels: val, x, y, t, xx, xy, yy, xt, yt,
gx (= psi_xxx + psi_xyy), gy (= psi_xxy + psi_yyy).  The Laplacian
contraction works because third derivatives only enter f_u/f_v via
u_xx+u_yy = d_y(Lap psi) and v_xx+v_yy = -d_x(Lap psi).

Per hidden layer, per supertile of 6x512 points:
  - PSUM: tri tiles [128,1536] (3 banks) x bufs=2 + pair [128,1024]:
    tri_a=(VAL,X,Y) tri_b=(T,XX,XY) tri_c=(YY,XT,YT) pair=(GX,GY)
  - ACT: tanh + batched PSUM->SBUF fp16 staging copies
  - DVE: wide fp16 2x-mode tensor_tensor with stride-0 broadcast APs
  - PE: 24 accumulating matmuls; scalar factors (-2,-4,-6,3,2...) folded
    into pre-scaled lhsT weight variants, so no scalar_tensor_tensor.
No GpSimd compute (Q7 sem ops cost ~2us each on cayman).
"""

import sys
from contextlib import ExitStack

import numpy as np

for _p in ("/opt/trn_rl_repo",):
    if _p not in sys.path:
        sys.path.insert(0, _p)

N_POINTS = 262144
N_CORES = 8
PPC = N_POINTS // N_CORES  # 32768
WIDTH = 20
N_HID = 7
G = 6
NPT = 512
SUPER = G * NPT            # 3072
NS = -(-PPC // SUPER)      # 11
KDIM = G * WIDTH           # 120
MPAD = 128                 # lhsT free dim padded for FWL

# channel ids
(VAL, CH_X, CH_Y, CH_T, CH_XX, CH_XY, CH_YY, CH_XT, CH_YT,
 CH_GX, CH_GY) = range(11)

# layer-1 ctile columns: (channel, which base: 0=f1,1=f2,2=f3)
L1_COLS = [(CH_X, 0), (CH_Y, 0), (CH_T, 0),
           (CH_XX, 1), (CH_XY, 1), (CH_YY, 1), (CH_XT, 1), (CH_YT, 1),
           (CH_GX, 2), (CH_GY, 2)]

# hidden weight variants: scale per slot
W_SCALES = [1.0, -2.0, -4.0, -6.0]
S_W, S_2W, S_4W, S_6W = 0, 1, 2, 3


def build_host_consts(W_in, b_in, W_hid, b_hid, W_out, b_out, lb, ub,
                      lambda_1, lambda_2):
    f32, f16 = np.float32, np.float16
    W_in = np.asarray(W_in, f32)
    b_in = np.asarray(b_in, f32)
    W_hid = np.asarray(W_hid, f32)
    b_hid = np.asarray(b_hid, f32)
    W_out = np.asarray(W_out, f32)
    b_out = np.asarray(b_out, f32)
    lb = np.asarray(lb, f32)
    ub = np.asarray(ub, f32)

    # fold normalization into layer 1
    s = (1.0 / (ub - lb)).astype(f32)
    c0 = (-lb * s).astype(f32)
    Wz = (W_in * s[:, None]).astype(f32)           # [3, 20]
    bz = (c0 @ W_in + b_in).astype(f32)            # [20]

    l1_lhsT = np.zeros((3 * G, MPAD), f32)
    hid_lhsT = np.zeros((N_HID, 4, KDIM, MPAD), f32)
    for g in range(G):
        l1_lhsT[3 * g:3 * g + 3, WIDTH * g:WIDTH * (g + 1)] = Wz
        for l in range(N_HID):
            for si, sc in enumerate(W_SCALES):
                hid_lhsT[l, si, WIDTH * g:WIDTH * (g + 1),
                         WIDTH * g:WIDTH * (g + 1)] = sc * W_hid[l]

    bias_tile = np.zeros((MPAD, 8), f32)
    bias_tile[:KDIM, 0] = np.tile(bz, G)
    for l in range(N_HID):
        bias_tile[:KDIM, 1 + l] = np.tile(b_hid[l], G)

    # layer-1 constants
    cx, cy, ct = Wz[0], Wz[1], Wz[2]
    cvecs = [cx, cy, ct,
             cx * cx, cx * cy, cy * cy, cx * ct, cy * ct,
             cx * (cx * cx + cy * cy), cy * (cx * cx + cy * cy)]
    c_tile = np.zeros((MPAD, 10), f32)
    c_tile[:KDIM] = np.stack([np.tile(v, G) for v in cvecs], axis=1)
    # layer-1 constants folded into h1 lhsT: lhsT_ch = diag(tile(c)) @ W1
    h1c = np.stack([np.tile(v, G)[:, None] * hid_lhsT[0, 0]
                    for v in cvecs])                    # [10, KDIM, MPAD]

    l1v = float(np.asarray(lambda_1).reshape(-1)[0])
    l2v = float(np.asarray(lambda_2).reshape(-1)[0])
    wpsi, wp = W_out[:, 0], W_out[:, 1]

    # output-stage lhsT bases per channel (rows at 0/32/64/96 quadrants)
    # po1: 0:12 [u,u] | 32:44 [v,v] | 64:76 [psi_xy, psi_xx]
    #      | 96:108 [psi_yy, psi_xy]
    # po2: 0:6 u | 32:38 v | 64:70 p | 96:108 [fu_lin, fv_lin]
    # fu_lin = psi_yt + p_x - l2*GY ; fv_lin = -psi_xt + p_y + l2*GX
    def base(entries):
        a = np.zeros((KDIM, MPAD), f32)
        for row0, vec in entries:
            for g in range(G):
                a[WIDTH * g:WIDTH * (g + 1), row0 + g] += vec
        return a

    o1b = {
        CH_X: base([(32, -wpsi), (38, -wpsi)]),
        CH_Y: base([(0, wpsi), (6, wpsi)]),
        CH_XX: base([(70, wpsi)]),
        CH_XY: base([(64, wpsi), (102, wpsi)]),
        CH_YY: base([(96, wpsi)]),
    }
    o2b = {
        VAL: base([(64, wp)]),
        CH_X: base([(32, -wpsi), (96, wp)]),
        CH_Y: base([(0, wpsi), (102, wp)]),
        CH_XT: base([(102, -wpsi)]),
        CH_YT: base([(96, wpsi)]),
        CH_GX: base([(102, l2v * wpsi)]),
        CH_GY: base([(96, -l2v * wpsi)]),
    }
    # piece-level lhsT lists (order must match build_program)
    o1_list = [o1b[CH_X], o1b[CH_Y],
               o1b[CH_XX], -2 * o1b[CH_XX],
               o1b[CH_XY], -2 * o1b[CH_XY],
               o1b[CH_YY], -2 * o1b[CH_YY]]
    o2_list = [o2b[VAL], o2b[CH_X], o2b[CH_Y],
               o2b[CH_XT], -2 * o2b[CH_XT],
               o2b[CH_YT], -2 * o2b[CH_YT],
               o2b[CH_GX], -6 * o2b[CH_GX], -4 * o2b[CH_GX],
               -2 * o2b[CH_GX],
               o2b[CH_GY], -2 * o2b[CH_GY], -4 * o2b[CH_GY],
               -6 * o2b[CH_GY]]

    lam_vec = np.zeros((12, 1), f32)
    lam_vec[0:6, 0] = l1v
    lam_vec[6:12, 0] = -l1v

    # batched weight layouts: single contiguous DMA each
    allw = np.concatenate([hid_lhsT.reshape(N_HID * 4, KDIM, MPAD), h1c])
    hidcat = np.ascontiguousarray(
        allw.transpose(1, 0, 2).reshape(KDIM, 38 * MPAD))
    ocat = np.ascontiguousarray(
        np.stack(o1_list + o2_list).transpose(1, 0, 2).reshape(KDIM, 23 * MPAD))
    return dict(l1_lhsT=l1_lhsT.astype(f16),
                hid_lhsT=hidcat.astype(f16),
                o_lhsT=ocat.astype(f16),
                bias_tile=bias_tile, c_tile=c_tile,
                lam_vec=lam_vec, p_bias=float(b_out[1]))


def build_program(p_bias, ns=NS):
    import concourse.bacc as bacc
    import concourse.bass as bass
    import concourse.tile as tile
    from concourse import mybir

    f32 = mybir.dt.float32
    f16 = mybir.dt.float16
    AF = mybir.ActivationFunctionType
    OP = mybir.AluOpType

    nc = bacc.Bacc("TRN2", target_bir_lowering=False, debug=False)

    xyz_d = nc.dram_tensor("xyz", [ns, 3 * G, NPT], f16, kind="ExternalInput")
    l1w_d = nc.dram_tensor("l1_lhsT", [3 * G, MPAD], f16,
                           kind="ExternalInput")
    hw_d = nc.dram_tensor("hid_lhsT", [KDIM, 38 * MPAD], f16,
                          kind="ExternalInput")
    o_d = nc.dram_tensor("o_lhsT", [KDIM, 23 * MPAD], f16,
                         kind="ExternalInput")
    bias_d = nc.dram_tensor("bias_tile", [MPAD, 8], f32,
                            kind="ExternalInput")
    c_d = nc.dram_tensor("c_tile", [MPAD, 10], f32, kind="ExternalInput")
    lam_d = nc.dram_tensor("lam_vec", [12, 1], f32, kind="ExternalInput")
    u_d = nc.dram_tensor("u_out", [ns, G, NPT], f32, kind="ExternalOutput")
    v_d = nc.dram_tensor("v_out", [ns, G, NPT], f32, kind="ExternalOutput")
    p_d = nc.dram_tensor("p_out", [ns, G, NPT], f32, kind="ExternalOutput")
    fu_d = nc.dram_tensor("fu_out", [ns, G, NPT], f32, kind="ExternalOutput")
    fv_d = nc.dram_tensor("fv_out", [ns, G, NPT], f32, kind="ExternalOutput")

    with tile.TileContext(nc) as tc, ExitStack() as ctx:
        dma = nc.sync.dma_start
        act = nc.scalar.activation
        tt = nc.vector.tensor_tensor
        gtt = nc.gpsimd.tensor_tensor
        ts = nc.vector.tensor_scalar
        stt = nc.vector.scalar_tensor_tensor
        mm = nc.tensor.matmul

        # ---- persistent weights ----
        wpool = ctx.enter_context(tc.tile_pool(name="wpool", bufs=1))
        l1w = wpool.tile([3 * G, MPAD], f16, name="l1w")
        dma(l1w[:], l1w_d[:])
        hwcat = wpool.tile([KDIM, 38 * MPAD], f16, name="hwcat")
        hws = [[hwcat[:, (l * 4 + si) * MPAD:(l * 4 + si + 1) * MPAD]
                for si in range(4)] for l in range(N_HID)]
        hws[0] = hws[0] + [hwcat[:, (28 + i) * MPAD:(29 + i) * MPAD]
                           for i in range(10)]
        ocat = wpool.tile([KDIM, 23 * MPAD], f16, name="ocat")
        o1w = [ocat[:, i * MPAD:(i + 1) * MPAD] for i in range(8)]
        o2w = [ocat[:, (8 + i) * MPAD:(9 + i) * MPAD] for i in range(15)]
        biases = wpool.tile([MPAD, 8], f32, name="biases")
        dma(biases[:], bias_d[:])
        ctile = wpool.tile([MPAD, 10], f32, name="ctile")
        dma(ctile[:], c_d[:])
        lam = wpool.tile([12, 1], f32, name="lam")
        dma(lam[:], lam_d[:])

        # ---- work pools ----
        xin = ctx.enter_context(tc.tile_pool(name="xin", bufs=2))
        wk = ctx.enter_context(tc.tile_pool(name="wk", bufs=3))
        wk1 = ctx.enter_context(tc.tile_pool(name="wk1", bufs=2))
        o12 = ctx.enter_context(tc.tile_pool(name="o12", bufs=2))
        o6 = ctx.enter_context(tc.tile_pool(name="o6", bufs=1))
        psT = ctx.enter_context(
            tc.tile_pool(name="psT", bufs=2, space=bass.MemorySpace.PSUM))
        psS = ctx.enter_context(
            tc.tile_pool(name="psS", bufs=2, space=bass.MemorySpace.PSUM))

        def view(ap, k):
            """[P, k*512] -> [P, k, 512]"""
            return ap.rearrange("p (k n) -> p k n", k=k)

        def mm_group(dst, W, pieces):
            for i, (ap, slot) in enumerate(pieces):
                mm(dst, W[slot], ap,
                   start=(i == 0), stop=(i == len(pieces) - 1))

        def hidden_layer(l, P):
            """P: dict ch -> list[(ap, slot)] of piece rhs APs + weight slot.
            Returns same for next layer."""
            W = hws[l]

            tri_a = psT.tile([MPAD, 3 * NPT], f32, name="tri")
            for s, ch in enumerate([VAL, CH_X, CH_Y]):
                mm_group(tri_a[:, s * NPT:(s + 1) * NPT], W, P[ch])
            tri_b = psT.tile([MPAD, 3 * NPT], f32, name="tri")
            for s, ch in enumerate([CH_T, CH_XX, CH_XY]):
                mm_group(tri_b[:, s * NPT:(s + 1) * NPT], W, P[ch])
            tri_c = psT.tile([MPAD, 3 * NPT], f32, name="tri")
            for s, ch in enumerate([CH_YY, CH_XT, CH_YT]):
                mm_group(tri_c[:, s * NPT:(s + 1) * NPT], W, P[ch])
            gx_ps = psS.tile([MPAD, NPT], f32, name="sg")
            mm_group(gx_ps[:], W, P[CH_GX])
            gy_ps = psS.tile([MPAD, NPT], f32, name="sg")
            mm_group(gy_ps[:], W, P[CH_GY])

            # ---- ACT staging ----
            ft = wk.tile([MPAD, 2 * NPT], f16, name="ft")   # [f1 | t0]
            act(ft[:, NPT:2 * NPT], tri_a[:, 0:NPT], AF.Tanh,
                bias=biases[:, 1 + l:2 + l])
            t0 = ft[:, NPT:2 * NPT]
            p2 = wk.tile([MPAD, NPT], f16, name="p2")
            act(p2[:], t0, AF.Square)
            qcat = wk.tile([MPAD, 3 * NPT], f16, name="qcat")
            act(qcat[:, 0:2 * NPT], tri_a[:, NPT:3 * NPT], AF.Copy,
                bias=0.0, scale=1.0)
            act(qcat[:, 2 * NPT:3 * NPT], tri_b[:, 0:NPT], AF.Copy,
                bias=0.0, scale=1.0)
            acat = wk.tile([MPAD, 7 * NPT], f16, name="acat")
            act(acat[:, 0:2 * NPT], tri_b[:, NPT:3 * NPT], AF.Copy,
                bias=0.0, scale=1.0)
            act(acat[:, 2 * NPT:5 * NPT], tri_c[:], AF.Copy,
                bias=0.0, scale=1.0)
            act(acat[:, 5 * NPT:6 * NPT], gx_ps[:], AF.Copy,
                bias=0.0, scale=1.0)
            act(acat[:, 6 * NPT:7 * NPT], gy_ps[:], AF.Copy,
                bias=0.0, scale=1.0)

            # ---- DVE ----
            f1 = ft[:, 0:NPT]
            ts(f1, p2[:], -1.0, 1.0, OP.mult, OP.add)
            qq = wk.tile([MPAD, NPT], f16, name="qq")
            act(qq[:], p2[:], AF.Copy, bias=-2.0, scale=6.0)
            # bw = [f1|t0] x [qx qy qt] -> [B1x B1y B1t | wtx wty wtt]
            bw = wk.tile([MPAD, 6 * NPT], f16, name="bw")
            tt(bw[:].rearrange("p (a b n) -> p a b n", a=2, b=3),
               view(ft[:], 2).unsqueeze(2).broadcast_to([MPAD, 2, 3, NPT]),
               view(qcat[:], 3).unsqueeze(1).broadcast_to([MPAD, 2, 3, NPT]),
               OP.mult)
            Bx, By = bw[:, 0:NPT], bw[:, NPT:2 * NPT]
            wt3 = bw[:, 3 * NPT:6 * NPT]
            # vall = f1 * acat (7 channels)
            vall = wk.tile([MPAD, 7 * NPT], f16, name="vall")
            tt(view(vall[:], 7),
               f1.unsqueeze(1).broadcast_to([MPAD, 7, NPT]),
               view(acat[:], 7), OP.mult)
            # cr1 = Bx * [wtx wty wtt] -> (XX XY XT)
            cr1 = wk.tile([MPAD, 3 * NPT], f16, name="cr1")
            tt(view(cr1[:], 3),
               Bx.unsqueeze(1).broadcast_to([MPAD, 3, NPT]),
               view(wt3, 3), OP.mult)
            # cr2 = By * [wty wtt] -> (YY YT)
            cr2 = wk.tile([MPAD, 2 * NPT], f16, name="cr2")
            tt(view(cr2[:], 2),
               By.unsqueeze(1).broadcast_to([MPAD, 2, NPT]),
               view(bw[:, 4 * NPT:6 * NPT], 2), OP.mult)
            # pcat = (Bx,By) * (qx,qy)
            pcat = wk.tile([MPAD, 2 * NPT], f16, name="pcat")
            tt(pcat[:], bw[:, 0:2 * NPT], qcat[:, 0:2 * NPT], OP.mult)
            psum = wk.tile([MPAD, NPT], f16, name="psum")
            tt(psum[:], pcat[:, 0:NPT], pcat[:, NPT:2 * NPT], OP.add)
            ssum = wk.tile([MPAD, NPT], f16, name="ssum")
            tt(ssum[:], qq[:], psum[:], OP.mult)
            rcat = wk.tile([MPAD, 2 * NPT], f16, name="rcat")
            tt(view(rcat[:], 2),
               ssum[:].unsqueeze(1).broadcast_to([MPAD, 2, NPT]),
               view(qcat[:, 0:2 * NPT], 2), OP.mult)
            # gcat[a,b] = v_a * wt_b, a in (xx,xy,yy), b in (x,y)
            gcat = wk.tile([MPAD, 6 * NPT], f16, name="gcat")
            tt(gcat[:].rearrange("p (a b n) -> p a b n", a=3, b=2),
               view(vall[:, 0:3 * NPT], 3).unsqueeze(2)
               .broadcast_to([MPAD, 3, 2, NPT]),
               view(bw[:, 3 * NPT:5 * NPT], 2).unsqueeze(1)
               .broadcast_to([MPAD, 3, 2, NPT]),
               OP.mult)

            def sl(tile_, k):
                return tile_[0:KDIM, k * NPT:(k + 1) * NPT]

            return {
                VAL: [(t0[0:KDIM, :], S_W)],
                CH_X: [(sl(bw, 0), S_W)],
                CH_Y: [(sl(bw, 1), S_W)],
                CH_T: [(sl(bw, 2), S_W)],
                CH_XX: [(sl(vall, 0), S_W), (sl(cr1, 0), S_2W)],
                CH_XY: [(sl(vall, 1), S_W), (sl(cr1, 1), S_2W)],
                CH_YY: [(sl(vall, 2), S_W), (sl(cr2, 0), S_2W)],
                CH_XT: [(sl(vall, 3), S_W), (sl(cr1, 2), S_2W)],
                CH_YT: [(sl(vall, 4), S_W), (sl(cr2, 1), S_2W)],
                CH_GX: [(sl(vall, 5), S_W), (sl(rcat, 0), S_W),
                        (sl(gcat, 0), S_6W), (sl(gcat, 3), S_4W),
                        (sl(gcat, 4), S_2W)],
                CH_GY: [(sl(vall, 6), S_W), (sl(rcat, 1), S_W),
                        (sl(gcat, 1), S_2W), (sl(gcat, 2), S_4W),
                        (sl(gcat, 5), S_6W)],
            }

        def l1_block(sidx):
            xt = xin.tile([3 * G, NPT], f16, name="xt")
            dma(xt[:], xyz_d[sidx])
            ps0 = psS.tile([MPAD, NPT], f32, name="sg")
            mm(ps0[:], l1w[:], xt[:], start=True, stop=True)
            ft1 = wk.tile([MPAD, 2 * NPT], f16, name="ft")
            act(ft1[:, NPT:2 * NPT], ps0[:], AF.Tanh,
                bias=biases[:, 0:1])
            t01 = ft1[:, NPT:2 * NPT]
            p21 = wk.tile([MPAD, NPT], f16, name="p2")
            tt(p21[:], t01, t01, OP.mult)
            f11 = ft1[:, 0:NPT]
            ts(f11, p21[:], -1.0, 1.0, OP.mult, OP.add)
            qq1 = wk.tile([MPAD, NPT], f16, name="qq")
            ts(qq1[:], p21[:], 6.0, -2.0, OP.mult, OP.add)
            mneg = wk1.tile([MPAD, NPT], f16, name="mneg")
            ts(mneg[:], t01, -2.0, None, OP.mult)
            ff2 = wk1.tile([MPAD, NPT], f16, name="ff2")
            tt(ff2[:], mneg[:], f11, OP.mult)
            ff3 = wk1.tile([MPAD, NPT], f16, name="ff3")
            tt(ff3[:], qq1[:], f11, OP.mult)
            srcs = {0: f11[0:KDIM, :], 1: ff2[0:KDIM, :], 2: ff3[0:KDIM, :]}
            P = {VAL: [(t01[0:KDIM, :], S_W)]}
            for k, (ch, b) in enumerate(L1_COLS):
                P[ch] = [(srcs[b], 4 + k)]
            return P

        def body(sidx, P):
            # ---- hidden layers ----
            for l in range(N_HID):
                P = hidden_layer(l, P)

            # ---- output stage ----
            po1t = psS.tile([MPAD, NPT], f32, name="sg")
            po2t = psS.tile([MPAD, NPT], f32, name="sg")
            po1 = po1t[:, 0:NPT]
            po2 = po2t[:, 0:NPT]
            p1_srcs = [P[CH_X][0], P[CH_Y][0],
                       P[CH_XX][0], P[CH_XX][1],
                       P[CH_XY][0], P[CH_XY][1],
                       P[CH_YY][0], P[CH_YY][1]]
            for i, (ap, _) in enumerate(p1_srcs):
                mm(po1, o1w[i], ap, start=(i == 0),
                   stop=(i == len(p1_srcs) - 1))
            # (piece, o2w index); GX/GY v- and r-pieces share base weights
            p2_srcs = [(P[VAL][0], 0), (P[CH_X][0], 1), (P[CH_Y][0], 2),
                       (P[CH_XT][0], 3), (P[CH_XT][1], 4),
                       (P[CH_YT][0], 5), (P[CH_YT][1], 6),
                       (P[CH_GX][0], 7), (P[CH_GX][1], 7),
                       (P[CH_GX][2], 8), (P[CH_GX][3], 9), (P[CH_GX][4], 10),
                       (P[CH_GY][0], 11), (P[CH_GY][1], 11),
                       (P[CH_GY][2], 12), (P[CH_GY][3], 13),
                       (P[CH_GY][4], 14)]
            for i, ((ap, _), wi) in enumerate(p2_srcs):
                mm(po2, o2w[wi], ap, start=(i == 0),
                   stop=(i == len(p2_srcs) - 1))

            a1 = o12.tile([12, NPT], f32, name="a1")
            act(a1[:], po1t[0:12, :], AF.Copy, bias=0.0, scale=1.0)
            a2 = o12.tile([12, NPT], f32, name="a2")
            act(a2[:], po1t[32:44, :], AF.Copy, bias=0.0, scale=1.0)
            pl1 = o12.tile([12, NPT], f32, name="pl1")
            tt(pl1[:], a1[:], po1t[64:76, :], OP.mult)
            pl2 = o12.tile([12, NPT], f32, name="pl2")
            tt(pl2[:], a2[:], po1t[96:108, :], OP.mult)
            dd = o12.tile([12, NPT], f32, name="dd")
            tt(dd[:], pl1[:], pl2[:], OP.add)
            ff = o12.tile([12, NPT], f32, name="ff")
            stt(ff[:], dd[:], lam[:], po2t[96:108, :],
                OP.mult, OP.add)
            pp = o6.tile([6, NPT], f32, name="pp")
            act(pp[:], po2t[64:70, :], AF.Copy,
                bias=float(p_bias), scale=1.0)
            uu = o6.tile([6, NPT], f32, name="uu")
            act(uu[:], po2t[0:6, :], AF.Copy, bias=0.0, scale=1.0)
            vv = o6.tile([6, NPT], f32, name="vv")
            act(vv[:], po2t[32:38, :], AF.Copy, bias=0.0, scale=1.0)

            dma(u_d[sidx], uu[:])
            dma(v_d[sidx], vv[:])
            dma(p_d[sidx], pp[:])
            dma(fu_d[sidx], ff[0:6, :])
            dma(fv_d[sidx], ff[6:12, :])

        prevP = None
        for s in range(ns):
            P1 = l1_block(s)
            if s == 0:
                dma(hwcat[:], hw_d[:])
                dma(ocat[:], o_d[:])
            if prevP is not None:
                body(s - 1, prevP)
            prevP = P1
        body(ns - 1, prevP)

    nc.compile()
    return nc


def make_in_maps(inputs, consts, ns=NS):
    x = np.asarray(inputs["x"], np.float32).reshape(-1)
    y = np.asarray(inputs["y"], np.float32).reshape(-1)
    t = np.asarray(inputs["t"], np.float32).reshape(-1)
    padpc = ns * SUPER
    shared = {k: consts[k] for k in ("l1_lhsT", "hid_lhsT", "o_lhsT",
                                     "bias_tile", "c_tile", "lam_vec")}
    in_maps = []
    for c in range(N_CORES):
        sl = slice(c * PPC, (c + 1) * PPC)

        def lay(vec):
            out = np.zeros((padpc,), np.float32)
            seg = vec[sl]
            out[:seg.shape[0]] = seg[:padpc]
            return out.reshape(ns, G, NPT)

        xyz = np.zeros((ns, 3 * G, NPT), np.float32)
        xyz[:, 0::3, :] = lay(x)
        xyz[:, 1::3, :] = lay(y)
        xyz[:, 2::3, :] = lay(t)
        in_maps.append({"xyz": xyz.astype(np.float16), **shared})
    return in_maps


def kernel(**inputs):
    consts = build_host_consts(
        inputs["W_in"], inputs["b_in"], inputs["W_hid"], inputs["b_hid"],
        inputs["W_out"], inputs["b_out"], inputs["lb"], inputs["ub"],
        inputs["lambda_1"], inputs["lambda_2"])
    nc = build_program(consts["p_bias"])
    in_maps = make_in_maps(inputs, consts)

    from concourse.bass_utils import run_bass_kernel_spmd
    res = run_bass_kernel_spmd(nc, in_maps, list(range(N_CORES)))

    outs = []
    for name in ("u_out", "v_out", "p_out", "fu_out", "fv_out"):
        full = np.concatenate(
            [np.asarray(res.results[c][name]).reshape(-1)[:PPC]
             for c in range(N_CORES)])
        outs.append(np.ascontiguousarray(full[:, None], dtype=np.float32))
    return tuple(outs)

